# revision 26
# baseline (speedup 1.0000x reference)
"""GCEncoder (RGCN basis-decomposition conv + mean aggregation + Dense/BN/ReLU)
as a Bass/Tile kernel on 8 Trainium2 NeuronCores — fp8 DoubleRow edition.

Math (reference):
  W[r]  = sum_b comp[r,b] * basis[b]                    [R, N, H0]
  h[r]  = x @ W[r]                                      [R, N, H0]
  agg[d] = sum_r (1/cnt[d,r]) * sum_{e: dst=d, type=r} h[r, src_e]
  feats = agg + x @ root + bias
  z     = feats @ fc_w.T ; per-row batchnorm over H1 + gamma/beta + relu
  out   = (z[:U], z[U:]) stacked -> [2, U, H1]

Device strategy (per core c of 8, 512 node-rows each).  The two heavy
contractions run in fp8-e4m3 with MatmulPerfMode.DoubleRow: each PE
instruction contracts TWO consecutive 128-deep k-tiles in the same ~500
cycles a bf16 instruction needs for one (2x effective throughput,
measured 256ns/instr cadence on hw):
  Phase A: h_c[r] = x_rows @ (SCL*W[r]), 16 k-tile-pairs per relation.
           PSUM result (= SCL*h) is quantized to fp8 and AllGathered per
           relation (overlapped with the remaining relations + root).
           Root part runs in bf16: rootf = x_rows @ root + bias.
  Phase B: per relation r, segment sums via dense indicator adjacency
           counts (exact small ints in fp8): psB = A_cnt.T @ h, 16
           k-tile-pairs over the gathered 4096 src rows.  Exact mean:
           feats += psB * (1/(SCL*cnt)) with per-partition scalars
           (scalar-engine Copy-activation scale + vector add).
  Phase C: feats -> PE-transpose -> z = feats @ fc_w.T (fp32) -> per-row
           BN (bn_stats/bn_aggr) + gamma/beta + ReLU.

fp8 rounding on hw matches ml_dtypes.float8_e4m3 bit-exactly (verified:
hw rel-err == host-sim rel-err to 5e-7), so the end-to-end error of this
scheme is deterministic at ~1.5e-2 for the fixed benchmark inputs.
"""
import numpy as np
import ml_dtypes

import concourse.bacc as bacc
import concourse.mybir as mybir
import concourse.tile as tile
from concourse.bass_utils import run_bass_kernel_spmd
from concourse.masks import make_identity

P = 128
NCORES = 8
N = 4096          # nodes
U = 2048          # users
R = 5             # relations
H0 = 500
H1 = 75
EPS = 1e-5
SCL = 32.0        # power-of-2 pre-scale on W so SCL*h fits fp8 range

NL = N // NCORES              # 512 node rows per core
KB = N // P                   # 32 contraction tiles over features
KP = KB // 2                  # 16 k-tile pairs (DoubleRow)
MB = NL // P                  # 4 M-tiles per core
QB = 4                        # H0 chunks for transpose/fc
QS = H0 // QB                 # 125

F32 = mybir.dt.float32
BF16 = mybir.dt.bfloat16
FP8 = mybir.dt.float8e4
DR = mybir.MatmulPerfMode.DoubleRow
NP8 = ml_dtypes.float8_e4m3

# test hooks
TRACE = False
LAST_RESULTS = None
_NC_CACHE = None


def _build():
    nc = bacc.Bacc("TRN2", target_bir_lowering=False, debug=False,
                   num_devices=NCORES)

    # host-swizzled inputs; layouts noted as [partition, free...]
    # xs[p, kb*NL + s] = fp8 x.T[kb*128+p, s@core]
    xs_d = nc.dram_tensor("xs", [P, KB * NL], FP8, kind="ExternalInput")
    # xb[p, kb*NL + s] = bf16 x.T[kb*128+p, s@core]   (root path)
    xb_d = nc.dram_tensor("xb", [P, KB * NL], BF16, kind="ExternalInput")
    # w8[p, (r*KB+kb)*H0 + j] = fp8(SCL*W[r][kb*128+p, j])
    w8_d = nc.dram_tensor("w8", [P, R * KB * H0], FP8, kind="ExternalInput")
    # rt[p, kb*H0 + j] = bf16 root[kb*128+p, j]
    rt_d = nc.dram_tensor("rt", [P, KB * H0], BF16, kind="ExternalInput")
    # a8[p, (((r*8+cb)*4+kt)*NL) + d] = edge count for
    #   (src=cb*512+kt*128+p, rel=r, dst=c*512+d) — exact small ints in fp8
    a8_d = nc.dram_tensor("a8", [P, R * NCORES * MB * NL], FP8,
                          kind="ExternalInput")
    # invc[p, m*R+r] = 1/(SCL*max(cnt[c*512+m*128+p, r], 1))
    invc_d = nc.dram_tensor("invc", [P, MB * R], F32, kind="ExternalInput")
    fcwt_d = nc.dram_tensor("fcwt", [H0, H1], F32, kind="ExternalInput")
    biasb_d = nc.dram_tensor("biasb", [P, H0], F32, kind="ExternalInput")
    gamma_d = nc.dram_tensor("gamma", [P, MB], F32, kind="ExternalInput")
    beta_d = nc.dram_tensor("beta", [P, MB], F32, kind="ExternalInput")
    out_d = nc.dram_tensor("out", [NL, H1], F32, kind="ExternalOutput")

    with tile.TileContext(nc) as tc:
        with (
            tc.tile_pool(name="big", bufs=1) as big,
            tc.tile_pool(name="slab", bufs=2) as slabp,
            tc.tile_pool(name="hq", bufs=4) as hqp,
            tc.tile_pool(name="aslab", bufs=6) as asp,
            tc.tile_pool(name="hslab", bufs=12) as hsp,
            tc.tile_pool(name="persist", bufs=4) as pp,
            tc.tile_pool(name="bn", bufs=4) as bnp,
            tc.tile_pool(name="ps", bufs=8, space="PSUM") as psp,
            tc.tile_pool(name="dram", bufs=1, space="DRAM") as dramp,
        ):
            # -------- input streams --------
            # first W slab chunk (scalar) + first x chunk (sync) gate the
            # first matmul: keep both small and first in their queues.
            CHUNKS = [(0, 4), (4, 8), (8, 16), (16, 32)]
            wsl0 = slabp.tile([P, KB, H0], FP8, tag="slab", name="wsl0")
            xs_sb = big.tile([P, KB, NL], FP8, tag="xs")
            for lo, hi in CHUNKS:
                nc.scalar.dma_start(
                    out=wsl0[:, lo:hi, :], in_=w8_d[:, lo * H0:hi * H0])
                nc.sync.dma_start(
                    out=xs_sb[:, lo:hi, :], in_=xs_d[:, lo * NL:hi * NL])
            # root weights: own persistent tile so the loads don't queue
            # behind phase A's slab-ring consumers
            rt_sb = big.tile([P, KB, H0], BF16, tag="rt")
            for ch in range(2):
                nc.scalar.dma_start(
                    out=rt_sb[:, ch * 16:(ch + 1) * 16, :],
                    in_=rt_d[:, ch * 16 * H0:(ch + 1) * 16 * H0],
                )
            # x bf16 (root path, needed at ~105us): split scalar/gpsimd so
            # the scalar preload backlog clears before root starts
            xb_sb = big.tile([P, KB, NL], BF16, tag="xb")
            nc.scalar.dma_start(
                out=xb_sb[:, 0:16, :], in_=xb_d[:, :16 * NL])
            nc.gpsimd.dma_start(
                out=xb_sb[:, 16:32, :], in_=xb_d[:, 16 * NL:32 * NL])

            h_cr = [dramp.tile([P, MB * H0], FP8, tag="h_c",
                               name=f"h_c{r}") for r in range(R)]
            h_ar = [dramp.tile([NCORES * P, MB * H0], FP8, tag="h_a",
                               addr_space="Shared", name=f"h_a{r}")
                    for r in range(R)]

            # -------- Phase A: h[r] = x @ SCL*W[r] (fp8 DR), AG per r ------
            for r in range(R):
                if r == 0:
                    wsl = wsl0
                else:
                    wsl = slabp.tile([P, KB, H0], FP8, tag="slab",
                                     name=f"wsl{r}")
                    for ch in range(2):
                        nc.sync.dma_start(
                            out=wsl[:, ch * 16:(ch + 1) * 16, :],
                            in_=w8_d[:, (r * KB + ch * 16) * H0:
                                     (r * KB + (ch + 1) * 16) * H0],
                        )
                ps = [psp.tile([P, H0], F32, tag="ps", name=f"psA_{r}_{m}")
                      for m in range(MB)]
                for kp in range(KP):
                    for m in range(MB):
                        nc.tensor.matmul(
                            ps[m],
                            xs_sb[:, 2 * kp:2 * kp + 2, m * P:(m + 1) * P],
                            wsl[:, 2 * kp:2 * kp + 2, :],
                            start=(kp == 0),
                            stop=(kp == KP - 1),
                            perf_mode=DR,
                        )
                for m in range(MB):
                    hhi = hqp.tile([P, H0], FP8, tag="hhi")
                    nc.vector.tensor_copy(out=hhi, in_=ps[m])
                    nc.gpsimd.dma_start(
                        out=h_cr[r][:, m * H0:(m + 1) * H0], in_=hhi)
                nc.gpsimd.collective_compute(
                    "AllGather",
                    mybir.AluOpType.bypass,
                    replica_groups=[list(range(NCORES))],
                    ins=[h_cr[r][:, :]],
                    outs=[h_ar[r][:, :]],
                )

            # -------- root path (bf16): rootf = x @ root + bias ------------
            biasb = big.tile([P, H0], F32, tag="bias")
            nc.scalar.dma_start(out=biasb, in_=biasb_d[:, :])
            invc = big.tile([P, MB * R], F32, tag="invc")
            nc.scalar.dma_start(out=invc, in_=invc_d[:, :])

            psR = [psp.tile([P, H0], F32, tag="ps", name=f"psR_{m}")
                   for m in range(MB)]
            for kb in range(KB):
                for m in range(MB):
                    nc.tensor.matmul(
                        psR[m],
                        xb_sb[:, kb, m * P:(m + 1) * P],
                        rt_sb[:, kb, :],
                        start=(kb == 0),
                        stop=(kb == KB - 1),
                    )
            feats = []
            for m in range(MB):
                f = pp.tile([P, H0], F32, tag="feats", name=f"feats_{m}")
                nc.vector.tensor_add(out=f, in0=psR[m], in1=biasb)
                feats.append(f)

            # phase C constants: load early, during the root window
            fcw_sb = big.tile([QS, QB, H1], F32, tag="fcw")
            nc.scalar.dma_start(
                out=fcw_sb,
                in_=fcwt_d[:, :].rearrange("(q p) j -> p q j", p=QS),
            )
            ident = big.tile([P, P], F32, tag="ident")
            make_identity(nc, ident)
            gam = big.tile([P, MB], F32, tag="gam")
            nc.scalar.dma_start(out=gam, in_=gamma_d[:, :])
            bet = big.tile([P, MB], F32, tag="bet")
            nc.scalar.dma_start(out=bet, in_=beta_d[:, :])
            eps_t = big.tile([P, 1], F32, tag="eps")
            nc.vector.memset(eps_t, EPS)

            # -------- Phase B: feats += (1/(SCL*cnt)) * A_cnt @ h ---------
            for r in range(R):
                psB = [psp.tile([P, H0], F32, tag="ps", name=f"psB_{r}_{m}")
                       for m in range(MB)]
                aa2 = []
                for cbp in range(NCORES // 2):
                    # two core-blocks per DMA: 4KB contiguous per partition
                    aa = asp.tile([P, 2, MB, NL], FP8, tag="aa")
                    base = ((r * NCORES + 2 * cbp) * MB) * NL
                    nc.sync.dma_start(
                        out=aa, in_=a8_d[:, base:base + 2 * MB * NL])
                    aa2.append(aa)
                for cb in range(NCORES):
                    hh = hsp.tile([P, MB, H0], FP8, tag="hh")
                    heng = nc.gpsimd if cb % 4 == 0 else nc.scalar
                    heng.dma_start(
                        out=hh, in_=h_ar[r][cb * P:(cb + 1) * P, :])
                    aa = aa2[cb // 2][:, cb % 2]
                    for tp in range(MB // 2):
                        for m in range(MB):
                            nc.tensor.matmul(
                                psB[m],
                                aa[:, 2 * tp:2 * tp + 2,
                                   m * P:(m + 1) * P],
                                hh[:, 2 * tp:2 * tp + 2, :],
                                start=(cb == 0 and tp == 0),
                                stop=(cb == NCORES - 1 and tp == MB // 2 - 1),
                                perf_mode=DR,
                            )
                for m in range(MB):
                    tmp = bnp.tile([P, H0], F32, tag="tmp")
                    nc.scalar.activation(
                        out=tmp, in_=psB[m],
                        func=mybir.ActivationFunctionType.Copy,
                        scale=invc[:, m * R + r:m * R + r + 1],
                    )
                    nc.vector.tensor_add(out=feats[m], in0=feats[m],
                                         in1=tmp)

            # -------- Phase C: feats -> fc -> BN -> ReLU ------------------
            fT = [pp.tile([P, NL], F32, tag="fT", name=f"fT_{q}")
                  for q in range(QB)]
            for m in range(MB):
                # per-m chain so m0's BN runs while m1 still drains
                for q in range(QB):
                    pt = psp.tile([P, P], F32, tag="ps", name=f"pt_{m}_{q}")
                    nc.tensor.transpose(
                        pt[:QS, :], feats[m][:, q * QS:(q + 1) * QS], ident
                    )
                    nc.vector.tensor_copy(
                        out=fT[q][:QS, m * P:(m + 1) * P], in_=pt[:QS, :]
                    )
                pz = psp.tile([P, H1], F32, tag="ps", name=f"pz_{m}")
                for q in range(QB):
                    nc.tensor.matmul(
                        pz,
                        fT[q][:QS, m * P:(m + 1) * P],
                        fcw_sb[:, q, :],
                        start=(q == 0),
                        stop=(q == QB - 1),
                    )
                stats = bnp.tile([P, 6], F32, tag="stats")
                nc.vector.bn_stats(out=stats, in_=pz)
                mv = bnp.tile([P, 2], F32, tag="mv")
                nc.vector.bn_aggr(out=mv, in_=stats)
                rstd = bnp.tile([P, 1], F32, tag="rstd")
                nc.scalar.activation(
                    out=rstd, in_=mv[:, 1:2],
                    func=mybir.ActivationFunctionType.Sqrt,
                    bias=eps_t, scale=1.0,
                )
                nc.vector.reciprocal(out=rstd, in_=rstd)
                g2 = bnp.tile([P, 1], F32, tag="g2")
                nc.vector.tensor_mul(out=g2, in0=rstd, in1=gam[:, m:m + 1])
                zt = bnp.tile([P, H1], F32, tag="zt")
                nc.vector.tensor_scalar(
                    out=zt, in0=pz,
                    scalar1=mv[:, 0:1], scalar2=g2,
                    op0=mybir.AluOpType.subtract, op1=mybir.AluOpType.mult,
                )
                nc.scalar.activation(
                    out=zt, in_=zt,
                    func=mybir.ActivationFunctionType.Relu,
                    bias=bet[:, m:m + 1], scale=1.0,
                )
                nc.scalar.dma_start(out=out_d[m * P:(m + 1) * P, :], in_=zt)

    nc.finalize()
    return nc


def _get_nc():
    global _NC_CACHE
    if _NC_CACHE is None:
        _NC_CACHE = _build()
    return _NC_CACHE


def _f8(a):
    return np.asarray(a, dtype=np.float32).astype(NP8)


def _prepare_in_maps(inputs) -> list:
    x = np.asarray(inputs["x"], dtype=np.float32)
    basis = np.asarray(inputs["basis"], dtype=np.float32)
    comp = np.asarray(inputs["comp"], dtype=np.float32)
    root = np.asarray(inputs["root"], dtype=np.float32)
    bias_rgcn = np.asarray(inputs["bias_rgcn"], dtype=np.float32)
    fc_w = np.asarray(inputs["fc_w"], dtype=np.float32)
    bn_gamma_u = np.asarray(inputs["bn_gamma_u"], dtype=np.float32)
    bn_beta_u = np.asarray(inputs["bn_beta_u"], dtype=np.float32)
    bn_gamma_i = np.asarray(inputs["bn_gamma_i"], dtype=np.float32)
    bn_beta_i = np.asarray(inputs["bn_beta_i"], dtype=np.float32)
    edge_index = np.asarray(inputs["edge_index"]).astype(np.int64)
    edge_type = np.asarray(inputs["edge_type"]).astype(np.int64)

    src, dst = edge_index[0], edge_index[1]
    et = edge_type

    # ---- x fp8 + bf16 copies (layout [p, kb, s]) ----
    xT = np.ascontiguousarray(x.T)                       # [feat, node]
    xs_full = (_f8(xT).reshape(KB, P, N)
               .transpose(1, 0, 2))                      # [p, kb, s]
    xb_full = (xT.astype(ml_dtypes.bfloat16)
               .reshape(KB, P, N).transpose(1, 0, 2))    # [p, kb, s]

    # ---- W = comp . basis, fp8 with SCL ----
    W = np.tensordot(comp, basis, axes=([1], [0]))       # [R, N, H0]
    w8 = _f8(W * SCL)                                    # [R, N, H0]
    w8 = np.ascontiguousarray(
        w8.reshape(R, KB, P, H0).transpose(2, 0, 1, 3)   # [p, r, kb, j]
        .reshape(P, R * KB * H0))
    rt = np.ascontiguousarray(
        root.astype(ml_dtypes.bfloat16)
        .reshape(KB, P, H0).transpose(1, 0, 2)           # [p, kb, j]
        .reshape(P, KB * H0))

    # ---- indicator adjacency counts (exact in fp8) + exact mean scale ----
    cnt = np.bincount(dst * R + et, minlength=N * R).reshape(N, R)
    lin = (et * N + src) * np.int64(N) + dst
    acnt = np.bincount(lin, minlength=R * N * N).astype(np.float32)
    a8_full = (acnt.astype(NP8)
               .reshape(R, NCORES, MB, P, N)             # [r, cb, kt, p, d]
               .transpose(3, 0, 1, 2, 4))                # [p, r, cb, kt, d]
    invc_full = (1.0 / (SCL * np.maximum(cnt, 1.0))).astype(np.float32)
    invc_full = (invc_full.reshape(NCORES, MB, P, R)     # [c, m, p, r]
                 .transpose(0, 2, 1, 3))                 # [c, p, m, r]

    fcwt = np.ascontiguousarray(fc_w.T)
    biasb = np.ascontiguousarray(
        np.broadcast_to(bias_rgcn, (P, H0)), dtype=np.float32)
    gamma_all = np.concatenate([bn_gamma_u, bn_gamma_i])
    beta_all = np.concatenate([bn_beta_u, bn_beta_i])

    in_maps = []
    for c in range(NCORES):
        sl = slice(c * NL, (c + 1) * NL)
        in_maps.append({
            "xs": np.ascontiguousarray(
                xs_full[:, :, sl]).reshape(P, KB * NL),
            "xb": np.ascontiguousarray(
                xb_full[:, :, sl]).reshape(P, KB * NL),
            "w8": w8,
            "rt": rt,
            "a8": np.ascontiguousarray(
                a8_full[:, :, :, :, sl]).reshape(P, R * NCORES * MB * NL),
            "invc": np.ascontiguousarray(
                invc_full[c]).reshape(P, MB * R),
            "fcwt": fcwt,
            "biasb": biasb,
            "gamma": np.ascontiguousarray(gamma_all[sl].reshape(MB, P).T),
            "beta": np.ascontiguousarray(beta_all[sl].reshape(MB, P).T),
        })
    return in_maps


def kernel(**inputs) -> np.ndarray:
    global LAST_RESULTS
    in_maps = _prepare_in_maps(inputs)
    nc = _get_nc()
    res = run_bass_kernel_spmd(
        nc, in_maps, core_ids=list(range(NCORES)), trace=TRACE,
    )
    LAST_RESULTS = res

    z = np.concatenate([res.results[c]["out"] for c in range(NCORES)], axis=0)
    return np.stack([z[:U], z[U:]], axis=0)


# revision 31
# speedup vs baseline: 1.0307x; 1.0307x over previous
"""GCEncoder (RGCN basis-decomposition conv + mean aggregation + Dense/BN/ReLU)
as a Bass/Tile kernel on 8 Trainium2 NeuronCores — fp8 DoubleRow edition.

Math (reference):
  W[r]  = sum_b comp[r,b] * basis[b]                    [R, N, H0]
  h[r]  = x @ W[r]                                      [R, N, H0]
  agg[d] = sum_r (1/cnt[d,r]) * sum_{e: dst=d, type=r} h[r, src_e]
  feats = agg + x @ root + bias
  z     = feats @ fc_w.T ; per-row batchnorm over H1 + gamma/beta + relu
  out   = (z[:U], z[U:]) stacked -> [2, U, H1]

Device strategy (per core c of 8, 512 node-rows each).  The two heavy
contractions run in fp8-e4m3 with MatmulPerfMode.DoubleRow: each PE
instruction contracts TWO consecutive 128-deep k-tiles in the same ~500
cycles a bf16 instruction needs for one (2x effective throughput,
measured 256ns/instr cadence on hw):
  Phase A: h_c[r] = x_rows @ (SCL*W[r]), 16 k-tile-pairs per relation.
           PSUM result (= SCL*h) is quantized to fp8 and AllGathered per
           relation (overlapped with the remaining relations + root).
           Root part runs in bf16: rootf = x_rows @ root + bias.
  Phase B: per relation r, segment sums via dense indicator adjacency
           counts (exact small ints in fp8): psB = A_cnt.T @ h, 16
           k-tile-pairs over the gathered 4096 src rows.  Exact mean:
           feats += psB * (1/(SCL*cnt)) with per-partition scalars
           (scalar-engine Copy-activation scale + vector add).
  Phase C: feats -> PE-transpose -> z = feats @ fc_w.T (fp32) -> per-row
           BN (bn_stats/bn_aggr) + gamma/beta + ReLU.

fp8 rounding on hw matches ml_dtypes.float8_e4m3 bit-exactly (verified:
hw rel-err == host-sim rel-err to 5e-7), so the end-to-end error of this
scheme is deterministic at ~1.5e-2 for the fixed benchmark inputs.
"""
import numpy as np
import ml_dtypes

import concourse.bacc as bacc
import concourse.mybir as mybir
import concourse.tile as tile
from concourse.bass_utils import run_bass_kernel_spmd
from concourse.masks import make_identity

P = 128
NCORES = 8
N = 4096          # nodes
U = 2048          # users
R = 5             # relations
H0 = 500
H1 = 75
EPS = 1e-5
SCL = 32.0        # power-of-2 pre-scale on W so SCL*h fits fp8 range

NL = N // NCORES              # 512 node rows per core
KB = N // P                   # 32 contraction tiles over features
KP = KB // 2                  # 16 k-tile pairs (DoubleRow)
MB = NL // P                  # 4 M-tiles per core
QB = 4                        # H0 chunks for transpose/fc
QS = H0 // QB                 # 125

F32 = mybir.dt.float32
BF16 = mybir.dt.bfloat16
FP8 = mybir.dt.float8e4
DR = mybir.MatmulPerfMode.DoubleRow
NP8 = ml_dtypes.float8_e4m3

# test hooks
TRACE = False
LAST_RESULTS = None
_NC_CACHE = None


def _build():
    nc = bacc.Bacc("TRN2", target_bir_lowering=False, debug=False,
                   num_devices=NCORES)

    # host-swizzled inputs; layouts noted as [partition, free...]
    # xs[p, kb*NL + s] = fp8 x.T[kb*128+p, s@core]
    xs_d = nc.dram_tensor("xs", [P, KB * NL], FP8, kind="ExternalInput")
    # xb[p, kb*NL + s] = bf16 x.T[kb*128+p, s@core]   (root path)
    xb_d = nc.dram_tensor("xb", [P, KB * NL], BF16, kind="ExternalInput")
    # w8[p, (r*KB+kb)*H0 + j] = fp8(SCL*W[r][kb*128+p, j])
    w8_d = nc.dram_tensor("w8", [P, R * KB * H0], FP8, kind="ExternalInput")
    # rt[p, kb*H0 + j] = bf16 root[kb*128+p, j]
    rt_d = nc.dram_tensor("rt", [P, KB * H0], BF16, kind="ExternalInput")
    # a8[p, (((r*8+cb)*4+kt)*NL) + d] = edge count for
    #   (src=cb*512+kt*128+p, rel=r, dst=c*512+d) — exact small ints in fp8
    a8_d = nc.dram_tensor("a8", [P, R * NCORES * MB * NL], FP8,
                          kind="ExternalInput")
    # invc[p, m*R+r] = 1/(SCL*max(cnt[c*512+m*128+p, r], 1))
    invc_d = nc.dram_tensor("invc", [P, MB * R], F32, kind="ExternalInput")
    fcwt_d = nc.dram_tensor("fcwt", [H0, H1], F32, kind="ExternalInput")
    biasb_d = nc.dram_tensor("biasb", [P, H0], F32, kind="ExternalInput")
    gamma_d = nc.dram_tensor("gamma", [P, MB], F32, kind="ExternalInput")
    beta_d = nc.dram_tensor("beta", [P, MB], F32, kind="ExternalInput")
    out_d = nc.dram_tensor("out", [NL, H1], F32, kind="ExternalOutput")

    with tile.TileContext(nc) as tc:
        with (
            tc.tile_pool(name="big", bufs=1) as big,
            tc.tile_pool(name="slab", bufs=2) as slabp,
            tc.tile_pool(name="hq", bufs=4) as hqp,
            tc.tile_pool(name="aslab", bufs=4) as asp,
            tc.tile_pool(name="hslab", bufs=12) as hsp,
            tc.tile_pool(name="persist", bufs=4) as pp,
            tc.tile_pool(name="bn", bufs=4) as bnp,
            tc.tile_pool(name="ps", bufs=8, space="PSUM") as psp,
            tc.tile_pool(name="dram", bufs=1, space="DRAM") as dramp,
        ):
            # -------- input streams --------
            # first W slab chunk (scalar) + first x chunk (sync) gate the
            # first matmul: keep both small and first in their queues.
            CHUNKS = [(0, 4), (4, 8), (8, 16), (16, 32)]
            wsl0 = slabp.tile([P, KB, H0], FP8, tag="slab", name="wsl0")
            xs_sb = big.tile([P, KB, NL], FP8, tag="xs")
            for lo, hi in CHUNKS:
                nc.scalar.dma_start(
                    out=wsl0[:, lo:hi, :], in_=w8_d[:, lo * H0:hi * H0])
                nc.sync.dma_start(
                    out=xs_sb[:, lo:hi, :], in_=xs_d[:, lo * NL:hi * NL])
            # root weights: own persistent tile so the loads don't queue
            # behind phase A's slab-ring consumers
            rt_sb = big.tile([P, KB, H0], BF16, tag="rt")
            for ch in range(2):
                nc.scalar.dma_start(
                    out=rt_sb[:, ch * 16:(ch + 1) * 16, :],
                    in_=rt_d[:, ch * 16 * H0:(ch + 1) * 16 * H0],
                )
            # x bf16 (root path, needed at ~105us): split scalar/gpsimd so
            # the scalar preload backlog clears before root starts
            xb_sb = big.tile([P, KB, NL], BF16, tag="xb")
            nc.scalar.dma_start(
                out=xb_sb[:, 0:16, :], in_=xb_d[:, :16 * NL])
            nc.gpsimd.dma_start(
                out=xb_sb[:, 16:32, :], in_=xb_d[:, 16 * NL:32 * NL])

            # relation-PAIR gather buffers: consumer-side hh loads then move
            # 4KB contiguous per partition (descriptor-rate friendly)
            NRP = (R + 1) // 2     # 3 pairs: (r0,r1), (r2,r3), (r4)
            PW = [min(2, R - 2 * rp) for rp in range(NRP)]   # pair widths
            h_cr = [dramp.tile([P, PW[rp] * MB * H0], FP8, tag="h_c",
                               name=f"h_c{rp}") for rp in range(NRP)]
            h_ar = [dramp.tile([NCORES * P, PW[rp] * MB * H0], FP8,
                               tag="h_a", addr_space="Shared",
                               name=f"h_a{rp}") for rp in range(NRP)]

            # -------- Phase A: h[r] = x @ SCL*W[r] (fp8 DR), AG per r ------
            for r in range(R):
                if r == 0:
                    wsl = wsl0
                else:
                    wsl = slabp.tile([P, KB, H0], FP8, tag="slab",
                                     name=f"wsl{r}")
                    for ch in range(2):
                        nc.sync.dma_start(
                            out=wsl[:, ch * 16:(ch + 1) * 16, :],
                            in_=w8_d[:, (r * KB + ch * 16) * H0:
                                     (r * KB + (ch + 1) * 16) * H0],
                        )
                ps = [psp.tile([P, H0], F32, tag="ps", name=f"psA_{r}_{m}")
                      for m in range(MB)]
                for kp in range(KP):
                    for m in range(MB):
                        nc.tensor.matmul(
                            ps[m],
                            xs_sb[:, 2 * kp:2 * kp + 2, m * P:(m + 1) * P],
                            wsl[:, 2 * kp:2 * kp + 2, :],
                            start=(kp == 0),
                            stop=(kp == KP - 1),
                            perf_mode=DR,
                        )
                for m in range(MB):
                    hhi = hqp.tile([P, H0], FP8, tag="hhi")
                    nc.vector.tensor_copy(out=hhi, in_=ps[m])
                    base = ((r % 2) * MB + m) * H0
                    nc.gpsimd.dma_start(
                        out=h_cr[r // 2][:, base:base + H0], in_=hhi)
                if r % 2 == 1 or r == R - 1:
                    rp = r // 2
                    nc.gpsimd.collective_compute(
                        "AllGather",
                        mybir.AluOpType.bypass,
                        replica_groups=[list(range(NCORES))],
                        ins=[h_cr[rp][:, :]],
                        outs=[h_ar[rp][:, :]],
                    )

            # -------- root path (bf16): rootf = x @ root + bias ------------
            biasb = big.tile([P, H0], F32, tag="bias")
            nc.scalar.dma_start(out=biasb, in_=biasb_d[:, :])
            invc = big.tile([P, MB * R], F32, tag="invc")
            nc.scalar.dma_start(out=invc, in_=invc_d[:, :])

            psR = [psp.tile([P, H0], F32, tag="ps", name=f"psR_{m}")
                   for m in range(MB)]
            for kb in range(KB):
                for m in range(MB):
                    nc.tensor.matmul(
                        psR[m],
                        xb_sb[:, kb, m * P:(m + 1) * P],
                        rt_sb[:, kb, :],
                        start=(kb == 0),
                        stop=(kb == KB - 1),
                    )
            feats = []
            for m in range(MB):
                f = pp.tile([P, H0], F32, tag="feats", name=f"feats_{m}")
                nc.vector.tensor_add(out=f, in0=psR[m], in1=biasb)
                feats.append(f)

            # phase C constants: load early, during the root window
            fcw_sb = big.tile([QS, QB, H1], F32, tag="fcw")
            nc.scalar.dma_start(
                out=fcw_sb,
                in_=fcwt_d[:, :].rearrange("(q p) j -> p q j", p=QS),
            )
            ident = big.tile([P, P], F32, tag="ident")
            make_identity(nc, ident)
            gam = big.tile([P, MB], F32, tag="gam")
            nc.scalar.dma_start(out=gam, in_=gamma_d[:, :])
            bet = big.tile([P, MB], F32, tag="bet")
            nc.scalar.dma_start(out=bet, in_=beta_d[:, :])
            eps_t = big.tile([P, 1], F32, tag="eps")
            nc.vector.memset(eps_t, EPS)

            # -------- Phase B: feats += (1/(SCL*cnt)) * A_cnt @ h ---------
            # hh tiles are loaded once per relation-PAIR (4KB/partition
            # descriptors, prefetchable as soon as that pair's AG lands)
            # and stay resident across both relations of the pair.
            for rp in range(NRP):
                pw = PW[rp]
                hhs = []
                for cb in range(NCORES):
                    hh = hsp.tile([P, pw, MB, H0], FP8, tag="hh",
                                  name=f"hh_{rp}_{cb}")
                    heng = nc.gpsimd if cb % 2 == 0 else nc.scalar
                    heng.dma_start(
                        out=hh, in_=h_ar[rp][cb * P:(cb + 1) * P, :])
                    hhs.append(hh)
                for rr in range(pw):
                    r = 2 * rp + rr
                    psB = [psp.tile([P, H0], F32, tag="ps",
                                    name=f"psB_{r}_{m}") for m in range(MB)]
                    aa2 = []
                    for cbp in range(NCORES // 2):
                        # two core-blocks per DMA: 4KB/partition contiguous
                        aa = asp.tile([P, 2, MB, NL], FP8, tag="aa")
                        base = ((r * NCORES + 2 * cbp) * MB) * NL
                        nc.sync.dma_start(
                            out=aa, in_=a8_d[:, base:base + 2 * MB * NL])
                        aa2.append(aa)
                    for cb in range(NCORES):
                        aa = aa2[cb // 2][:, cb % 2]
                        hh = hhs[cb]
                        for tp in range(MB // 2):
                            for m in range(MB):
                                nc.tensor.matmul(
                                    psB[m],
                                    aa[:, 2 * tp:2 * tp + 2,
                                       m * P:(m + 1) * P],
                                    hh[:, rr, 2 * tp:2 * tp + 2, :],
                                    start=(cb == 0 and tp == 0),
                                    stop=(cb == NCORES - 1
                                          and tp == MB // 2 - 1),
                                    perf_mode=DR,
                                )
                    for m in range(MB):
                        tmp = bnp.tile([P, H0], F32, tag="tmp")
                        nc.scalar.activation(
                            out=tmp, in_=psB[m],
                            func=mybir.ActivationFunctionType.Copy,
                            scale=invc[:, m * R + r:m * R + r + 1],
                        )
                        nc.vector.tensor_add(out=feats[m], in0=feats[m],
                                             in1=tmp)

            # -------- Phase C: feats -> fc -> BN -> ReLU ------------------
            fT = [pp.tile([P, NL], F32, tag="fT", name=f"fT_{q}")
                  for q in range(QB)]
            for m in range(MB):
                # per-m chain so m0's BN runs while m1 still drains
                for q in range(QB):
                    pt = psp.tile([P, P], F32, tag="ps", name=f"pt_{m}_{q}")
                    nc.tensor.transpose(
                        pt[:QS, :], feats[m][:, q * QS:(q + 1) * QS], ident
                    )
                    nc.vector.tensor_copy(
                        out=fT[q][:QS, m * P:(m + 1) * P], in_=pt[:QS, :]
                    )
                pz = psp.tile([P, H1], F32, tag="ps", name=f"pz_{m}")
                for q in range(QB):
                    nc.tensor.matmul(
                        pz,
                        fT[q][:QS, m * P:(m + 1) * P],
                        fcw_sb[:, q, :],
                        start=(q == 0),
                        stop=(q == QB - 1),
                    )
                stats = bnp.tile([P, 6], F32, tag="stats")
                nc.vector.bn_stats(out=stats, in_=pz)
                mv = bnp.tile([P, 2], F32, tag="mv")
                nc.vector.bn_aggr(out=mv, in_=stats)
                rstd = bnp.tile([P, 1], F32, tag="rstd")
                nc.scalar.activation(
                    out=rstd, in_=mv[:, 1:2],
                    func=mybir.ActivationFunctionType.Sqrt,
                    bias=eps_t, scale=1.0,
                )
                nc.vector.reciprocal(out=rstd, in_=rstd)
                g2 = bnp.tile([P, 1], F32, tag="g2")
                nc.vector.tensor_mul(out=g2, in0=rstd, in1=gam[:, m:m + 1])
                zt = bnp.tile([P, H1], F32, tag="zt")
                nc.vector.tensor_scalar(
                    out=zt, in0=pz,
                    scalar1=mv[:, 0:1], scalar2=g2,
                    op0=mybir.AluOpType.subtract, op1=mybir.AluOpType.mult,
                )
                nc.scalar.activation(
                    out=zt, in_=zt,
                    func=mybir.ActivationFunctionType.Relu,
                    bias=bet[:, m:m + 1], scale=1.0,
                )
                nc.scalar.dma_start(out=out_d[m * P:(m + 1) * P, :], in_=zt)

    nc.finalize()
    return nc


def _get_nc():
    global _NC_CACHE
    if _NC_CACHE is None:
        _NC_CACHE = _build()
    return _NC_CACHE


def _f8(a):
    return np.asarray(a, dtype=np.float32).astype(NP8)


def _prepare_in_maps(inputs) -> list:
    x = np.asarray(inputs["x"], dtype=np.float32)
    basis = np.asarray(inputs["basis"], dtype=np.float32)
    comp = np.asarray(inputs["comp"], dtype=np.float32)
    root = np.asarray(inputs["root"], dtype=np.float32)
    bias_rgcn = np.asarray(inputs["bias_rgcn"], dtype=np.float32)
    fc_w = np.asarray(inputs["fc_w"], dtype=np.float32)
    bn_gamma_u = np.asarray(inputs["bn_gamma_u"], dtype=np.float32)
    bn_beta_u = np.asarray(inputs["bn_beta_u"], dtype=np.float32)
    bn_gamma_i = np.asarray(inputs["bn_gamma_i"], dtype=np.float32)
    bn_beta_i = np.asarray(inputs["bn_beta_i"], dtype=np.float32)
    edge_index = np.asarray(inputs["edge_index"]).astype(np.int64)
    edge_type = np.asarray(inputs["edge_type"]).astype(np.int64)

    src, dst = edge_index[0], edge_index[1]
    et = edge_type

    # ---- x fp8 + bf16 copies (layout [p, kb, s]) ----
    xT = np.ascontiguousarray(x.T)                       # [feat, node]
    xs_full = (_f8(xT).reshape(KB, P, N)
               .transpose(1, 0, 2))                      # [p, kb, s]
    xb_full = (xT.astype(ml_dtypes.bfloat16)
               .reshape(KB, P, N).transpose(1, 0, 2))    # [p, kb, s]

    # ---- W = comp . basis, fp8 with SCL ----
    W = np.tensordot(comp, basis, axes=([1], [0]))       # [R, N, H0]
    w8 = _f8(W * SCL)                                    # [R, N, H0]
    w8 = np.ascontiguousarray(
        w8.reshape(R, KB, P, H0).transpose(2, 0, 1, 3)   # [p, r, kb, j]
        .reshape(P, R * KB * H0))
    rt = np.ascontiguousarray(
        root.astype(ml_dtypes.bfloat16)
        .reshape(KB, P, H0).transpose(1, 0, 2)           # [p, kb, j]
        .reshape(P, KB * H0))

    # ---- indicator adjacency counts (exact in fp8) + exact mean scale ----
    cnt = np.bincount(dst * R + et, minlength=N * R).reshape(N, R)
    lin = (et * N + src) * np.int64(N) + dst
    acnt = np.bincount(lin, minlength=R * N * N).astype(np.float32)
    a8_full = (acnt.astype(NP8)
               .reshape(R, NCORES, MB, P, N)             # [r, cb, kt, p, d]
               .transpose(3, 0, 1, 2, 4))                # [p, r, cb, kt, d]
    invc_full = (1.0 / (SCL * np.maximum(cnt, 1.0))).astype(np.float32)
    invc_full = (invc_full.reshape(NCORES, MB, P, R)     # [c, m, p, r]
                 .transpose(0, 2, 1, 3))                 # [c, p, m, r]

    fcwt = np.ascontiguousarray(fc_w.T)
    biasb = np.ascontiguousarray(
        np.broadcast_to(bias_rgcn, (P, H0)), dtype=np.float32)
    gamma_all = np.concatenate([bn_gamma_u, bn_gamma_i])
    beta_all = np.concatenate([bn_beta_u, bn_beta_i])

    in_maps = []
    for c in range(NCORES):
        sl = slice(c * NL, (c + 1) * NL)
        in_maps.append({
            "xs": np.ascontiguousarray(
                xs_full[:, :, sl]).reshape(P, KB * NL),
            "xb": np.ascontiguousarray(
                xb_full[:, :, sl]).reshape(P, KB * NL),
            "w8": w8,
            "rt": rt,
            "a8": np.ascontiguousarray(
                a8_full[:, :, :, :, sl]).reshape(P, R * NCORES * MB * NL),
            "invc": np.ascontiguousarray(
                invc_full[c]).reshape(P, MB * R),
            "fcwt": fcwt,
            "biasb": biasb,
            "gamma": np.ascontiguousarray(gamma_all[sl].reshape(MB, P).T),
            "beta": np.ascontiguousarray(beta_all[sl].reshape(MB, P).T),
        })
    return in_maps


def kernel(**inputs) -> np.ndarray:
    global LAST_RESULTS
    in_maps = _prepare_in_maps(inputs)
    nc = _get_nc()
    res = run_bass_kernel_spmd(
        nc, in_maps, core_ids=list(range(NCORES)), trace=TRACE,
    )
    LAST_RESULTS = res

    z = np.concatenate([res.results[c]["out"] for c in range(NCORES)], axis=0)
    return np.stack([z[:U], z[U:]], axis=0)
